# revision 8
# baseline (speedup 1.0000x reference)
"""Bilinear LTN scoring kernel for Trainium2 (8 NeuronCores).

scores[i] = h_emb[h[i]]^T @ W[r[i]] @ t_emb[t[i]],  B=4096, DIM=256.

Strategy: the batch shares only N_REL=500 relation matrices (256KB each),
so items are grouped by relation and the *relations* are sharded across the
8 cores. Each relation matrix is then streamed from HBM exactly once
system-wide (~16MB/core) instead of once per item (131MB/core).

Per core (all static shapes):
  - groups of <=32 items (relation chunks), G=64 group slots, 2048 item slots
  - entity rows for h/t are gathered on-device via indirect DMA
    (pad slots use an out-of-bounds index and are skipped by the DMA)
  - H is transposed on the PE (identity matmul) into [dim, slot] layout
  - per group g: U[i,b] = sum_a H[i,a] W_g[a,b] via 2 accumulating matmuls
    (float32r, moving dim 256 -> full PE rate)
  - fused multiply-reduce against gathered T rows gives the 32 scores
"""

import sys

for _p in ("/opt/trn_rl_repo",):
    if _p not in sys.path:
        sys.path.insert(0, _p)

import numpy as np

import concourse.bass as bass
import concourse.mybir as mybir
import concourse.tile as tile
from concourse.bass import IndirectOffsetOnAxis
from concourse.bass_utils import run_bass_kernel_spmd
from concourse.masks import make_identity
from concourse.vector_clock import ScopedClock

DIM = 256
N_ENT = 100000
N_REL = 500
NCORES = 8
C = 32            # items per group (matmul stationary width)
G = 64            # group slots per core
BLOCKS = G // 4   # 4 groups share one [128, 256] PSUM tile
SLOTS = G * C     # 2048 item slots per core
IDX_COLS = SLOTS // 128  # 16
PAD_IDX = 0x7FFF0000     # > N_ENT-1 -> indirect DMA skips the row

F32 = mybir.dt.float32
F32R = mybir.dt.float32r
I32 = mybir.dt.int32
MM_DT = F32  # matmul operand dtype (F32 exact; bf16 = faster, lossier)

_MAX_DRAIN_WAITS = 1


def _install_drain_fix():
    """This container's walrus accepts only one sync wait on the Tile exit
    Drain; split the extra waits onto preceding NOPs."""
    if getattr(tile.TileContext, "_drain_fix_installed", False):
        return

    def _split_multi_waits(nc):
        """Split any instruction carrying more than one sync wait into
        preceding same-engine NOPs each carrying one wait."""
        cur_bb = nc.cur_bb.bb
        for f in nc.m.functions:
            for blk in f.blocks:
                bb = blk if hasattr(blk, "instructions") else blk.bb
                i = 0
                while i < len(bb.instructions):
                    inst = bb.instructions[i]
                    si = getattr(inst, "sync_info", None)
                    waits = list(si.on_wait or []) if si is not None else []
                    if len(waits) > _MAX_DRAIN_WAITS:
                        si.on_wait = waits[-_MAX_DRAIN_WAITS:]
                        extra = waits[: -_MAX_DRAIN_WAITS]
                        nops = []
                        for w0 in range(0, len(extra), _MAX_DRAIN_WAITS):
                            nop_inst = nc.engines[inst.engine].nop(
                                nofuse=True, hint="wait_split"
                            )
                            nop_inst.ins.sync_info = mybir.SyncInfo(
                                on_wait=extra[w0 : w0 + _MAX_DRAIN_WAITS],
                                on_update=[],
                            )
                            nops.append(nop_inst.ins)
                        for n in nops:
                            cur_bb.instructions.remove(n)
                        for j, n in enumerate(nops):
                            bb.instructions.insert(i + j, n)
                        i += len(nops)
                    i += 1

    def _drain_and_barrier(self, tick_clock, wait_clock):
        drain_inst = self.nc.sync.drain()
        wait_clock.add_sem_waits(
            drain_inst.ins, ScopedClock({None: tick_clock.global_clock})
        )
        self.nc.all_engine_barrier()
        assert self.sems is not None
        popped = self.nc._tile_sem_poison_stack.pop()
        assert popped is self._sem_poison
        self.nc.clear_and_free_semaphores(list(self.sems.allocated().values()))
        self.nc.all_engine_barrier()
        _split_multi_waits(self.nc)

    tile.TileContext._drain_and_barrier = _drain_and_barrier
    tile.TileContext._drain_fix_installed = True


def _build_bass():
    _install_drain_fix()
    nc = bass.Bass()
    ent = nc.declare_dram_parameter("ent", [N_ENT, DIM], F32, isOutput=False)
    wrows = nc.declare_dram_parameter("wrows", [G, DIM * DIM], MM_DT, isOutput=False)
    hidx = nc.declare_dram_parameter("hidx", [128, IDX_COLS], I32, isOutput=False)
    tidx = nc.declare_dram_parameter("tidx", [128, IDX_COLS], I32, isOutput=False)
    out = nc.declare_dram_parameter("out", [128, IDX_COLS], F32, isOutput=True)

    with tile.TileContext(nc) as tc:
        with (
            tc.tile_pool(name="const", bufs=1) as const_pool,
            tc.tile_pool(name="gather", bufs=1) as gather_pool,
            tc.tile_pool(name="ht", bufs=1) as ht_pool,
            tc.tile_pool(name="w", bufs=4) as w_pool,
            tc.tile_pool(name="scratch", bufs=2) as scratch_pool,
            tc.tile_pool(name="upsum", bufs=6, space="PSUM") as u_pool,
            tc.tile_pool(name="trpsum", bufs=2, space="PSUM") as tr_pool,
        ):
            ident = const_pool.tile([128, 128], F32, tag="ident")
            make_identity(nc, ident[:])

            hidx_t = const_pool.tile([128, IDX_COLS], I32, tag="hidx")
            tidx_t = const_pool.tile([128, IDX_COLS], I32, tag="tidx")
            nc.sync.dma_start(out=hidx_t[:], in_=hidx[:])
            nc.sync.dma_start(out=tidx_t[:], in_=tidx[:])

            out_sb = const_pool.tile([128, IDX_COLS], F32, tag="outsb")

            # gather tiles: block j holds slots [128j, 128j+128)
            hg = []
            tg = []
            for j in range(IDX_COLS):
                hg.append(gather_pool.tile([128, DIM], F32, tag=f"hg{j}", name=f"hg{j}"))
                tg.append(gather_pool.tile([128, DIM], F32, tag=f"tg{j}", name=f"tg{j}"))
            for j in range(IDX_COLS):
                nc.gpsimd.indirect_dma_start(
                    out=hg[j][:],
                    out_offset=None,
                    in_=ent[:],
                    in_offset=IndirectOffsetOnAxis(ap=hidx_t[:, j : j + 1], axis=0),
                    bounds_check=N_ENT - 1,
                    oob_is_err=False,
                )
                nc.gpsimd.indirect_dma_start(
                    out=tg[j][:],
                    out_offset=None,
                    in_=ent[:],
                    in_offset=IndirectOffsetOnAxis(ap=tidx_t[:, j : j + 1], axis=0),
                    bounds_check=N_ENT - 1,
                    oob_is_err=False,
                )

            # transpose H into [a, slot] layout: ht[k][j][a_sub, p] =
            # H[slot=128j+p][dim=128k+a_sub]
            ht = [[None] * IDX_COLS for _ in range(2)]
            for j in range(IDX_COLS):
                for k in range(2):
                    trp = tr_pool.tile([128, 128], F32, space="PSUM")
                    nc.tensor.transpose(
                        out=trp[:],
                        in_=hg[j][:, k * 128 : (k + 1) * 128],
                        identity=ident[:],
                    )
                    htt = ht_pool.tile([128, 128], MM_DT, tag=f"ht{k}_{j}", name=f"ht{k}_{j}")
                    ht[k][j] = htt
                    # alternate copy engine to split PSUM->SBUF traffic
                    if (j * 2 + k) % 2 == 0:
                        nc.vector.tensor_copy(htt[:], trp[:])
                    else:
                        nc.scalar.copy(htt[:], trp[:])

            # relation-matrix stream + per-group matmuls + fused reduce
            ups = None
            for m in range(G // 2):  # 2 groups per DMA
                wt = w_pool.tile([128, 1024], MM_DT)
                nc.sync.dma_start(
                    out=wt[:].rearrange("p (g k b) -> p g k b", g=2, k=2, b=DIM),
                    in_=wrows[2 * m : 2 * m + 2].rearrange(
                        "g (k a b) -> a g k b", k=2, a=128, b=DIM
                    ),
                )
                if m % 2 == 0:
                    ups = u_pool.tile([128, DIM], F32, space="PSUM")
                for d in range(2):
                    g = 2 * m + d
                    j = g // 4
                    part = 32 * (g % 4)
                    for k in range(2):
                        nc.tensor.matmul(
                            out=ups[part : part + 32, :],
                            lhsT=ht[k][j][:, 32 * (g % 4) : 32 * (g % 4) + 32],
                            rhs=wt[:, (d * 2 + k) * DIM : (d * 2 + k + 1) * DIM],
                            start=(k == 0),
                            stop=(k == 1),
                            tile_position=(0, part),
                        )
                if m % 2 == 1:
                    j = m // 2
                    sc = scratch_pool.tile([128, DIM], F32)
                    nc.vector.tensor_tensor(
                        out=sc[:],
                        in0=ups[:],
                        in1=tg[j][:],
                        op=mybir.AluOpType.mult,
                    )
                    nc.vector.tensor_reduce(
                        out=out_sb[:, j : j + 1],
                        in_=sc[:],
                        axis=mybir.AxisListType.X,
                        op=mybir.AluOpType.add,
                    )

            nc.sync.dma_start(out=out[:], in_=out_sb[:])

    return nc


_NC_CACHE = None


def _get_nc():
    global _NC_CACHE
    if _NC_CACHE is None:
        _NC_CACHE = _build_bass()
    return _NC_CACHE


def _pack(h, r, t, ent_weight, rel_weight):
    """Group items by relation, chunk to <=C, balance chunks across cores."""
    B = h.shape[0]
    order = np.argsort(r, kind="stable")
    rs = r[order]
    starts = np.flatnonzero(np.r_[True, rs[1:] != rs[:-1]])
    ends = np.r_[starts[1:], len(rs)]
    chunks = []  # (rel_id, item_positions)
    for s0, e0 in zip(starts, ends):
        rid = int(rs[s0])
        for c0 in range(s0, e0, C):
            chunks.append((rid, order[c0 : min(c0 + C, e0)]))
    chunks.sort(key=lambda x: -len(x[1]))

    per_core = [[] for _ in range(NCORES)]
    counts = [0] * NCORES
    items = [0] * NCORES
    for ch in chunks:
        k = min(range(NCORES), key=lambda q: (counts[q], items[q]))
        per_core[k].append(ch)
        counts[k] += 1
        items[k] += len(ch[1])
    assert max(counts) <= G, f"chunk overflow: {counts}"

    in_maps = []
    slot_maps = []  # per core: (slot_array, item_positions_array)
    for k in range(NCORES):
        wrows = np.zeros((G, DIM * DIM), dtype=np.float32)
        hidx = np.full((128, IDX_COLS), PAD_IDX, dtype=np.int32)
        tidx = np.full((128, IDX_COLS), PAD_IDX, dtype=np.int32)
        slots = []
        positions = []
        for g, (rid, pos) in enumerate(per_core[k]):
            wrows[g] = rel_weight[rid]
            s = g * C + np.arange(len(pos))
            hidx[s % 128, s // 128] = h[pos]
            tidx[s % 128, s // 128] = t[pos]
            slots.append(s)
            positions.append(pos)
        slots = np.concatenate(slots) if slots else np.zeros(0, np.int64)
        positions = (
            np.concatenate(positions) if positions else np.zeros(0, np.int64)
        )
        slot_maps.append((slots, positions))
        in_maps.append(
            {
                "ent": ent_weight,
                "wrows": wrows,
                "hidx": hidx,
                "tidx": tidx,
            }
        )
    return in_maps, slot_maps


def _run(h, r, t, ent_weight, rel_weight, trace=False):
    h = np.asarray(h).astype(np.int64)
    r = np.asarray(r).astype(np.int64)
    t = np.asarray(t).astype(np.int64)
    ent_weight = np.ascontiguousarray(np.asarray(ent_weight, dtype=np.float32))
    rel_weight = np.ascontiguousarray(np.asarray(rel_weight, dtype=np.float32))
    assert ent_weight.shape == (N_ENT, DIM)
    assert rel_weight.shape == (N_REL, DIM * DIM)

    in_maps, slot_maps = _pack(h, r, t, ent_weight, rel_weight)
    nc = _get_nc()
    res = run_bass_kernel_spmd(
        nc, in_maps, core_ids=list(range(NCORES)), trace=trace
    )
    scores = np.empty(h.shape[0], dtype=np.float32)
    for k in range(NCORES):
        o = res.results[k]["out"]
        slots, positions = slot_maps[k]
        scores[positions] = o[slots % 128, slots // 128]
    return scores, res


def kernel(h, r, t, ent_weight, rel_weight):
    scores, _ = _run(h, r, t, ent_weight, rel_weight, trace=False)
    return scores


# revision 9
# speedup vs baseline: 1.0867x; 1.0867x over previous
"""Bilinear LTN scoring kernel for Trainium2 (8 NeuronCores).

scores[i] = h_emb[h[i]]^T @ W[r[i]] @ t_emb[t[i]],  B=4096, DIM=256.

Strategy: the batch shares only N_REL=500 relation matrices (256KB each),
so items are grouped by relation and the *relations* are sharded across the
8 cores. Each relation matrix is then streamed from HBM exactly once
system-wide (~16MB/core) instead of once per item (131MB/core).

Per core (all static shapes):
  - groups of <=32 items (relation chunks), G=64 group slots, 2048 item slots
  - entity rows for h/t are gathered on-device via indirect DMA
    (pad slots use an out-of-bounds index and are skipped by the DMA)
  - H is transposed on the PE (identity matmul) into [dim, slot] layout
  - per group g: U[i,b] = sum_a H[i,a] W_g[a,b] via accumulating matmuls
  - fused multiply-reduce against gathered T rows gives the 32 scores

Precision modes (MODE):
  f32    — exact fp32 matmuls (4 cyc/row on the PE, slow)
  bf16   — all matmul operands bf16 (1 cyc/row; ~2e-3 scale-relative error)
  bf16x2 — W and H split into bf16 hi+lo, 3 cross terms accumulated in
           PSUM: error ~1e-6, at bf16 streaming rate
"""

import sys

for _p in ("/opt/trn_rl_repo",):
    if _p not in sys.path:
        sys.path.insert(0, _p)

import ml_dtypes
import numpy as np

import concourse.bass as bass
import concourse.mybir as mybir
import concourse.tile as tile
from concourse.bass import IndirectOffsetOnAxis
from concourse.bass_utils import run_bass_kernel_spmd
from concourse.masks import make_identity
from concourse.vector_clock import ScopedClock

DIM = 256
N_ENT = 100000
N_REL = 500
NCORES = 8
C = 32            # items per group (matmul stationary width)
G = 64            # group slots per core
SLOTS = G * C     # 2048 item slots per core
IDX_COLS = SLOTS // 128  # 16
PAD_IDX = 0x7FFF0000     # > N_ENT-1 -> indirect DMA skips the row

F32 = mybir.dt.float32
BF16 = mybir.dt.bfloat16
I32 = mybir.dt.int32

MODE = "bf16x2"  # "f32" | "bf16" | "bf16x2"

_MAX_WAITS = 1


def _install_walrus_fixes():
    """This container's walrus accepts only one sync wait per instruction;
    split extra waits onto preceding same-engine NOPs."""
    if getattr(tile.TileContext, "_drain_fix_installed", False):
        return

    def _split_multi_waits(nc):
        cur_bb = nc.cur_bb.bb
        for f in nc.m.functions:
            for blk in f.blocks:
                bb = blk if hasattr(blk, "instructions") else blk.bb
                i = 0
                while i < len(bb.instructions):
                    inst = bb.instructions[i]
                    si = getattr(inst, "sync_info", None)
                    waits = list(si.on_wait or []) if si is not None else []
                    if len(waits) > _MAX_WAITS:
                        si.on_wait = waits[-_MAX_WAITS:]
                        extra = waits[: -_MAX_WAITS]
                        nops = []
                        for w0 in range(0, len(extra), _MAX_WAITS):
                            nop_inst = nc.engines[inst.engine].nop(
                                nofuse=True, hint="wait_split"
                            )
                            nop_inst.ins.sync_info = mybir.SyncInfo(
                                on_wait=extra[w0 : w0 + _MAX_WAITS],
                                on_update=[],
                            )
                            nops.append(nop_inst.ins)
                        for n in nops:
                            cur_bb.instructions.remove(n)
                        for j, n in enumerate(nops):
                            bb.instructions.insert(i + j, n)
                        i += len(nops)
                    i += 1

    def _drain_and_barrier(self, tick_clock, wait_clock):
        drain_inst = self.nc.sync.drain()
        wait_clock.add_sem_waits(
            drain_inst.ins, ScopedClock({None: tick_clock.global_clock})
        )
        self.nc.all_engine_barrier()
        assert self.sems is not None
        popped = self.nc._tile_sem_poison_stack.pop()
        assert popped is self._sem_poison
        self.nc.clear_and_free_semaphores(list(self.sems.allocated().values()))
        self.nc.all_engine_barrier()
        _split_multi_waits(self.nc)

    tile.TileContext._drain_and_barrier = _drain_and_barrier
    tile.TileContext._drain_fix_installed = True


def _build_bass(mode):
    _install_walrus_fixes()
    mm_dt = F32 if mode == "f32" else BF16
    nterms = {"f32": 1, "bf16": 1, "bf16x2": 3}[mode]
    # W streams: f32/bf16 -> one (W); bf16x2 -> two (Whi, Wlo)
    nw = 2 if mode == "bf16x2" else 1

    nc = bass.Bass()
    ent = nc.declare_dram_parameter("ent", [N_ENT, DIM], F32, isOutput=False)
    wrows = nc.declare_dram_parameter("wrows", [nw * G, DIM * DIM], mm_dt, isOutput=False)
    hidx = nc.declare_dram_parameter("hidx", [128, IDX_COLS], I32, isOutput=False)
    tidx = nc.declare_dram_parameter("tidx", [128, IDX_COLS], I32, isOutput=False)
    out = nc.declare_dram_parameter("out", [128, IDX_COLS], F32, isOutput=True)

    with tile.TileContext(nc) as tc:
        with (
            tc.tile_pool(name="const", bufs=1) as const_pool,
            tc.tile_pool(name="gather", bufs=1) as gather_pool,
            tc.tile_pool(name="ht", bufs=1) as ht_pool,
            tc.tile_pool(name="w", bufs=4) as w_pool,
            tc.tile_pool(name="scratch", bufs=2) as scratch_pool,
            tc.tile_pool(name="upsum", bufs=6, space="PSUM") as u_pool,
            tc.tile_pool(name="trpsum", bufs=2, space="PSUM") as tr_pool,
        ):
            ident = const_pool.tile([128, 128], F32, tag="ident")
            make_identity(nc, ident[:])

            hidx_t = const_pool.tile([128, IDX_COLS], I32, tag="hidx")
            tidx_t = const_pool.tile([128, IDX_COLS], I32, tag="tidx")
            nc.sync.dma_start(out=hidx_t[:], in_=hidx[:])
            nc.sync.dma_start(out=tidx_t[:], in_=tidx[:])

            out_sb = const_pool.tile([128, IDX_COLS], F32, tag="outsb")

            # gather tiles: block j holds slots [128j, 128j+128)
            hg = []
            tg = []
            for j in range(IDX_COLS):
                hg.append(gather_pool.tile([128, DIM], F32, tag=f"hg{j}", name=f"hg{j}"))
                tg.append(gather_pool.tile([128, DIM], F32, tag=f"tg{j}", name=f"tg{j}"))
            for j in range(IDX_COLS):
                nc.gpsimd.indirect_dma_start(
                    out=hg[j][:],
                    out_offset=None,
                    in_=ent[:],
                    in_offset=IndirectOffsetOnAxis(ap=hidx_t[:, j : j + 1], axis=0),
                    bounds_check=N_ENT - 1,
                    oob_is_err=False,
                )
                nc.gpsimd.indirect_dma_start(
                    out=tg[j][:],
                    out_offset=None,
                    in_=ent[:],
                    in_offset=IndirectOffsetOnAxis(ap=tidx_t[:, j : j + 1], axis=0),
                    bounds_check=N_ENT - 1,
                    oob_is_err=False,
                )

            # transpose H into [a, slot] layout: ht[k][j][a_sub, p] =
            # H[slot=128j+p][dim=128k+a_sub]; for bf16x2 also build the
            # low-order residual Hlo = f32(H) - f32(bf16(H)).
            ht = [[None] * IDX_COLS for _ in range(2)]
            htlo = [[None] * IDX_COLS for _ in range(2)] if mode == "bf16x2" else None
            for j in range(IDX_COLS):
                for k in range(2):
                    trp = tr_pool.tile([128, 128], F32, space="PSUM")
                    nc.tensor.transpose(
                        out=trp[:],
                        in_=hg[j][:, k * 128 : (k + 1) * 128],
                        identity=ident[:],
                    )
                    htt = ht_pool.tile(
                        [128, 128], mm_dt, tag=f"ht{k}_{j}", name=f"ht{k}_{j}"
                    )
                    ht[k][j] = htt
                    if (j * 2 + k) % 2 == 0:
                        nc.vector.tensor_copy(htt[:], trp[:])
                    else:
                        nc.scalar.copy(htt[:], trp[:])
                    if mode == "bf16x2":
                        # residual: back-cast hi to f32, subtract, round to bf16
                        res32 = scratch_pool.tile(
                            [128, 128], F32, tag="res32", name="res32"
                        )
                        nc.vector.tensor_tensor(
                            out=res32[:],
                            in0=trp[:],
                            in1=htt[:],
                            op=mybir.AluOpType.subtract,
                        )
                        htl = ht_pool.tile(
                            [128, 128], BF16, tag=f"htlo{k}_{j}", name=f"htlo{k}_{j}"
                        )
                        htlo[k][j] = htl
                        nc.scalar.copy(htl[:], res32[:])

            # relation-matrix stream + per-group matmuls + fused reduce
            # wrows rows: [hi rows 0..G) , lo rows G..2G) for bf16x2
            ups = None
            for m in range(G // 2):  # 2 groups per DMA per stream
                wts = []
                for s in range(nw):
                    wt = w_pool.tile(
                        [128, 1024], mm_dt, tag=f"wt{s}", name=f"wt{s}_{m}"
                    )
                    nc.sync.dma_start(
                        out=wt[:].rearrange("p (g k b) -> p g k b", g=2, k=2, b=DIM),
                        in_=wrows[
                            s * G + 2 * m : s * G + 2 * m + 2
                        ].rearrange("g (k a b) -> a g k b", k=2, a=128, b=DIM),
                    )
                    wts.append(wt)
                if m % 2 == 0:
                    ups = u_pool.tile([128, DIM], F32, space="PSUM")
                for d in range(2):
                    g = 2 * m + d
                    j = g // 4
                    part = 32 * (g % 4)
                    # accumulation terms: (lhs, rhs_stream)
                    if mode == "bf16x2":
                        terms = [
                            (ht[0][j], wts[0], 0),
                            (ht[1][j], wts[0], 1),
                            (ht[0][j], wts[1], 0),
                            (ht[1][j], wts[1], 1),
                            (htlo[0][j], wts[0], 0),
                            (htlo[1][j], wts[0], 1),
                        ]
                    else:
                        terms = [(ht[0][j], wts[0], 0), (ht[1][j], wts[0], 1)]
                    nt = len(terms)
                    for ti, (lhs_tile, wt, k) in enumerate(terms):
                        nc.tensor.matmul(
                            out=ups[part : part + 32, :],
                            lhsT=lhs_tile[:, 32 * (g % 4) : 32 * (g % 4) + 32],
                            rhs=wt[:, (d * 2 + k) * DIM : (d * 2 + k + 1) * DIM],
                            start=(ti == 0),
                            stop=(ti == nt - 1),
                            tile_position=(0, part),
                        )
                if m % 2 == 1:
                    j = m // 2
                    sc = scratch_pool.tile([128, DIM], F32, tag="sc", name=f"sc{m}")
                    nc.vector.tensor_tensor(
                        out=sc[:],
                        in0=ups[:],
                        in1=tg[j][:],
                        op=mybir.AluOpType.mult,
                    )
                    nc.vector.tensor_reduce(
                        out=out_sb[:, j : j + 1],
                        in_=sc[:],
                        axis=mybir.AxisListType.X,
                        op=mybir.AluOpType.add,
                    )

            nc.sync.dma_start(out=out[:], in_=out_sb[:])

    return nc


_NC_CACHE = {}


def _get_nc(mode):
    if mode not in _NC_CACHE:
        _NC_CACHE[mode] = _build_bass(mode)
    return _NC_CACHE[mode]


def _pack(h, r, t, ent_weight, rel_weight, mode):
    """Group items by relation, chunk to <=C, balance chunks across cores."""
    order = np.argsort(r, kind="stable")
    rs = r[order]
    starts = np.flatnonzero(np.r_[True, rs[1:] != rs[:-1]])
    ends = np.r_[starts[1:], len(rs)]
    chunks = []  # (rel_id, item_positions)
    for s0, e0 in zip(starts, ends):
        rid = int(rs[s0])
        for c0 in range(s0, e0, C):
            chunks.append((rid, order[c0 : min(c0 + C, e0)]))
    chunks.sort(key=lambda x: -len(x[1]))

    per_core = [[] for _ in range(NCORES)]
    counts = [0] * NCORES
    items = [0] * NCORES
    for ch in chunks:
        k = min(range(NCORES), key=lambda q: (counts[q], items[q]))
        per_core[k].append(ch)
        counts[k] += 1
        items[k] += len(ch[1])
    assert max(counts) <= G, f"chunk overflow: {counts}"

    if mode == "f32":
        def wcast(x):
            return x
        wdt = np.float32
    else:
        def wcast(x):
            return x.astype(ml_dtypes.bfloat16)
        wdt = ml_dtypes.bfloat16
    nw = 2 if mode == "bf16x2" else 1

    in_maps = []
    slot_maps = []  # per core: (slot_array, item_positions_array)
    for k in range(NCORES):
        wrows = np.zeros((nw * G, DIM * DIM), dtype=wdt)
        hidx = np.full((128, IDX_COLS), PAD_IDX, dtype=np.int32)
        tidx = np.full((128, IDX_COLS), PAD_IDX, dtype=np.int32)
        slots = []
        positions = []
        for g, (rid, pos) in enumerate(per_core[k]):
            w32 = rel_weight[rid]
            whi = wcast(w32)
            wrows[g] = whi
            if nw == 2:
                wrows[G + g] = wcast(w32 - whi.astype(np.float32))
            s = g * C + np.arange(len(pos))
            hidx[s % 128, s // 128] = h[pos]
            tidx[s % 128, s // 128] = t[pos]
            slots.append(s)
            positions.append(pos)
        slots = np.concatenate(slots) if slots else np.zeros(0, np.int64)
        positions = (
            np.concatenate(positions) if positions else np.zeros(0, np.int64)
        )
        slot_maps.append((slots, positions))
        in_maps.append(
            {"ent": ent_weight, "wrows": wrows, "hidx": hidx, "tidx": tidx}
        )
    return in_maps, slot_maps


def _run(h, r, t, ent_weight, rel_weight, trace=False, mode=None):
    if mode is None:
        mode = MODE
    h = np.asarray(h).astype(np.int64)
    r = np.asarray(r).astype(np.int64)
    t = np.asarray(t).astype(np.int64)
    ent_weight = np.ascontiguousarray(np.asarray(ent_weight, dtype=np.float32))
    rel_weight = np.ascontiguousarray(np.asarray(rel_weight, dtype=np.float32))
    assert ent_weight.shape == (N_ENT, DIM)
    assert rel_weight.shape == (N_REL, DIM * DIM)

    in_maps, slot_maps = _pack(h, r, t, ent_weight, rel_weight, mode)
    nc = _get_nc(mode)
    res = run_bass_kernel_spmd(
        nc, in_maps, core_ids=list(range(NCORES)), trace=trace
    )
    scores = np.empty(h.shape[0], dtype=np.float32)
    for k in range(NCORES):
        o = res.results[k]["out"]
        slots, positions = slot_maps[k]
        scores[positions] = o[slots % 128, slots // 128]
    return scores, res


def kernel(h, r, t, ent_weight, rel_weight):
    scores, _ = _run(h, r, t, ent_weight, rel_weight, trace=False)
    return scores


# revision 10
# speedup vs baseline: 1.2094x; 1.1129x over previous
"""Bilinear LTN scoring kernel for Trainium2 (8 NeuronCores).

scores[i] = h_emb[h[i]]^T @ W[r[i]] @ t_emb[t[i]],  B=4096, DIM=256.

Strategy: the batch shares only N_REL=500 relation matrices (256KB each),
so items are grouped by relation and the *relations* are sharded across the
8 cores. Each relation matrix is then streamed from HBM exactly once
system-wide (~16MB/core) instead of once per item (131MB/core).

Per core (all static shapes):
  - groups of <=32 items (relation chunks), G=64 group slots, 2048 item slots
  - entity rows for h/t are gathered on-device via indirect DMA
    (pad slots use an out-of-bounds index and are skipped by the DMA)
  - H is transposed on the PE (identity matmul) into [dim, slot] layout
  - per group g: U[i,b] = sum_a H[i,a] W_g[a,b] via accumulating matmuls
  - fused multiply-reduce against gathered T rows gives the 32 scores

Precision modes (MODE):
  f32    — exact fp32 matmuls (4 cyc/row on the PE, slow)
  bf16   — all matmul operands bf16 (1 cyc/row; ~2e-3 scale-relative error)
  bf16x2 — W and H split into bf16 hi+lo, 3 cross terms accumulated in
           PSUM: error ~1e-6, at bf16 streaming rate
"""

import sys

for _p in ("/opt/trn_rl_repo",):
    if _p not in sys.path:
        sys.path.insert(0, _p)

import ml_dtypes
import numpy as np

import concourse.bass as bass
import concourse.mybir as mybir
import concourse.tile as tile
from concourse.bass import IndirectOffsetOnAxis
from concourse.bass_utils import run_bass_kernel_spmd
from concourse.masks import make_identity
from concourse.vector_clock import ScopedClock

DIM = 256
N_ENT = 100000
N_REL = 500
NCORES = 8
C = 32            # items per group (matmul stationary width)
G = 64            # group slots per core
SLOTS = G * C     # 2048 item slots per core
IDX_COLS = SLOTS // 128  # 16
PAD_IDX = 0x7FFF0000     # > N_ENT-1 -> indirect DMA skips the row

F32 = mybir.dt.float32
BF16 = mybir.dt.bfloat16
I32 = mybir.dt.int32

MODE = "bf16x2"  # "f32" | "bf16" | "bf16x2"

_MAX_WAITS = 1


def _install_walrus_fixes():
    """This container's walrus accepts only one sync wait per instruction;
    split extra waits onto preceding same-engine NOPs."""
    if getattr(tile.TileContext, "_drain_fix_installed", False):
        return

    def _split_multi_waits(nc):
        cur_bb = nc.cur_bb.bb
        for f in nc.m.functions:
            for blk in f.blocks:
                bb = blk if hasattr(blk, "instructions") else blk.bb
                i = 0
                while i < len(bb.instructions):
                    inst = bb.instructions[i]
                    si = getattr(inst, "sync_info", None)
                    waits = list(si.on_wait or []) if si is not None else []
                    if len(waits) > _MAX_WAITS:
                        si.on_wait = waits[-_MAX_WAITS:]
                        extra = waits[: -_MAX_WAITS]
                        nops = []
                        for w0 in range(0, len(extra), _MAX_WAITS):
                            nop_inst = nc.engines[inst.engine].nop(
                                nofuse=True, hint="wait_split"
                            )
                            nop_inst.ins.sync_info = mybir.SyncInfo(
                                on_wait=extra[w0 : w0 + _MAX_WAITS],
                                on_update=[],
                            )
                            nops.append(nop_inst.ins)
                        for n in nops:
                            cur_bb.instructions.remove(n)
                        for j, n in enumerate(nops):
                            bb.instructions.insert(i + j, n)
                        i += len(nops)
                    i += 1

    def _drain_and_barrier(self, tick_clock, wait_clock):
        drain_inst = self.nc.sync.drain()
        wait_clock.add_sem_waits(
            drain_inst.ins, ScopedClock({None: tick_clock.global_clock})
        )
        self.nc.all_engine_barrier()
        assert self.sems is not None
        popped = self.nc._tile_sem_poison_stack.pop()
        assert popped is self._sem_poison
        self.nc.clear_and_free_semaphores(list(self.sems.allocated().values()))
        self.nc.all_engine_barrier()
        _split_multi_waits(self.nc)

    tile.TileContext._drain_and_barrier = _drain_and_barrier
    tile.TileContext._drain_fix_installed = True


def _build_bass(mode):
    _install_walrus_fixes()
    mm_dt = F32 if mode == "f32" else BF16
    nterms = {"f32": 1, "bf16": 1, "bf16x2": 3}[mode]
    # W streams: f32/bf16 -> one (W); bf16x2 -> two (Whi, Wlo)
    nw = 2 if mode == "bf16x2" else 1

    nc = bass.Bass()
    ent = nc.declare_dram_parameter("ent", [N_ENT, DIM], F32, isOutput=False)
    wrows = nc.declare_dram_parameter("wrows", [nw * G, DIM * DIM], mm_dt, isOutput=False)
    hidx = nc.declare_dram_parameter("hidx", [128, IDX_COLS], I32, isOutput=False)
    tidx = nc.declare_dram_parameter("tidx", [128, IDX_COLS], I32, isOutput=False)
    out = nc.declare_dram_parameter("out", [128, IDX_COLS], F32, isOutput=True)

    with tile.TileContext(nc) as tc:
        with (
            tc.tile_pool(name="const", bufs=1) as const_pool,
            tc.tile_pool(name="gather", bufs=1) as gather_pool,
            tc.tile_pool(name="ht", bufs=1) as ht_pool,
            tc.tile_pool(name="w", bufs=10) as w_pool,
            tc.tile_pool(name="scratch", bufs=2) as scratch_pool,
            tc.tile_pool(name="upsum", bufs=6, space="PSUM") as u_pool,
            tc.tile_pool(name="trpsum", bufs=2, space="PSUM") as tr_pool,
        ):
            ident = const_pool.tile([128, 128], F32, tag="ident")
            make_identity(nc, ident[:])

            hidx_t = const_pool.tile([128, IDX_COLS], I32, tag="hidx")
            tidx_t = const_pool.tile([128, IDX_COLS], I32, tag="tidx")
            nc.sync.dma_start(out=hidx_t[:], in_=hidx[:])
            nc.sync.dma_start(out=tidx_t[:], in_=tidx[:])

            out_sb = const_pool.tile([128, IDX_COLS], F32, tag="outsb")

            # gather tiles: block j holds slots [128j, 128j+128)
            hg = []
            tg = []
            for j in range(IDX_COLS):
                hg.append(gather_pool.tile([128, DIM], F32, tag=f"hg{j}", name=f"hg{j}"))
                tg.append(gather_pool.tile([128, DIM], F32, tag=f"tg{j}", name=f"tg{j}"))
            for j in range(IDX_COLS):
                nc.gpsimd.indirect_dma_start(
                    out=hg[j][:],
                    out_offset=None,
                    in_=ent[:],
                    in_offset=IndirectOffsetOnAxis(ap=hidx_t[:, j : j + 1], axis=0),
                    bounds_check=N_ENT - 1,
                    oob_is_err=False,
                )
            for j in range(IDX_COLS):
                nc.gpsimd.indirect_dma_start(
                    out=tg[j][:],
                    out_offset=None,
                    in_=ent[:],
                    in_offset=IndirectOffsetOnAxis(ap=tidx_t[:, j : j + 1], axis=0),
                    bounds_check=N_ENT - 1,
                    oob_is_err=False,
                )

            # transpose H into [a, slot] layout: ht[k][j][a_sub, p] =
            # H[slot=128j+p][dim=128k+a_sub]; for bf16x2 also build the
            # low-order residual Hlo = f32(H) - f32(bf16(H)).
            ht = [[None] * IDX_COLS for _ in range(2)]
            htlo = [[None] * IDX_COLS for _ in range(2)] if mode == "bf16x2" else None
            for j in range(IDX_COLS):
                for k in range(2):
                    trp = tr_pool.tile([128, 128], F32, space="PSUM")
                    nc.tensor.transpose(
                        out=trp[:],
                        in_=hg[j][:, k * 128 : (k + 1) * 128],
                        identity=ident[:],
                    )
                    htt = ht_pool.tile(
                        [128, 128], mm_dt, tag=f"ht{k}_{j}", name=f"ht{k}_{j}"
                    )
                    ht[k][j] = htt
                    if (j * 2 + k) % 2 == 0:
                        nc.vector.tensor_copy(htt[:], trp[:])
                    else:
                        nc.scalar.copy(htt[:], trp[:])
                    if mode == "bf16x2":
                        # residual: back-cast hi to f32, subtract, round to bf16
                        res32 = scratch_pool.tile(
                            [128, 128], F32, tag="res32", name="res32"
                        )
                        nc.vector.tensor_tensor(
                            out=res32[:],
                            in0=trp[:],
                            in1=htt[:],
                            op=mybir.AluOpType.subtract,
                        )
                        htl = ht_pool.tile(
                            [128, 128], BF16, tag=f"htlo{k}_{j}", name=f"htlo{k}_{j}"
                        )
                        htlo[k][j] = htl
                        nc.scalar.copy(htl[:], res32[:])

            # relation-matrix stream + per-group matmuls + fused reduce
            # wrows rows: [hi rows 0..G) , lo rows G..2G) for bf16x2
            ups = None
            for m in range(G // 2):  # 2 groups per DMA per stream
                wts = []
                for s in range(nw):
                    wt = w_pool.tile(
                        [128, 1024], mm_dt, tag=f"wt{s}", name=f"wt{s}_{m}"
                    )
                    dma_eng = nc.sync if (m + s) % 2 == 0 else nc.scalar
                    dma_eng.dma_start(
                        out=wt[:].rearrange("p (g k b) -> p g k b", g=2, k=2, b=DIM),
                        in_=wrows[
                            s * G + 2 * m : s * G + 2 * m + 2
                        ].rearrange("g (k a b) -> a g k b", k=2, a=128, b=DIM),
                    )
                    wts.append(wt)
                if m % 2 == 0:
                    ups = u_pool.tile([128, DIM], F32, space="PSUM")
                for d in range(2):
                    g = 2 * m + d
                    j = g // 4
                    part = 32 * (g % 4)
                    # accumulation terms: (lhs, rhs_stream)
                    if mode == "bf16x2":
                        terms = [
                            (ht[0][j], wts[0], 0),
                            (ht[1][j], wts[0], 1),
                            (ht[0][j], wts[1], 0),
                            (ht[1][j], wts[1], 1),
                            (htlo[0][j], wts[0], 0),
                            (htlo[1][j], wts[0], 1),
                        ]
                    else:
                        terms = [(ht[0][j], wts[0], 0), (ht[1][j], wts[0], 1)]
                    nt = len(terms)
                    for ti, (lhs_tile, wt, k) in enumerate(terms):
                        nc.tensor.matmul(
                            out=ups[part : part + 32, :],
                            lhsT=lhs_tile[:, 32 * (g % 4) : 32 * (g % 4) + 32],
                            rhs=wt[:, (d * 2 + k) * DIM : (d * 2 + k + 1) * DIM],
                            start=(ti == 0),
                            stop=(ti == nt - 1),
                            tile_position=(0, part),
                        )
                if m % 2 == 1:
                    j = m // 2
                    sc = scratch_pool.tile([128, DIM], F32, tag="sc", name=f"sc{m}")
                    nc.vector.tensor_tensor(
                        out=sc[:],
                        in0=ups[:],
                        in1=tg[j][:],
                        op=mybir.AluOpType.mult,
                    )
                    nc.vector.tensor_reduce(
                        out=out_sb[:, j : j + 1],
                        in_=sc[:],
                        axis=mybir.AxisListType.X,
                        op=mybir.AluOpType.add,
                    )

            nc.sync.dma_start(out=out[:], in_=out_sb[:])

    return nc


_NC_CACHE = {}


def _get_nc(mode):
    if mode not in _NC_CACHE:
        _NC_CACHE[mode] = _build_bass(mode)
    return _NC_CACHE[mode]


def _pack(h, r, t, ent_weight, rel_weight, mode):
    """Group items by relation, chunk to <=C, balance chunks across cores."""
    order = np.argsort(r, kind="stable")
    rs = r[order]
    starts = np.flatnonzero(np.r_[True, rs[1:] != rs[:-1]])
    ends = np.r_[starts[1:], len(rs)]
    chunks = []  # (rel_id, item_positions)
    for s0, e0 in zip(starts, ends):
        rid = int(rs[s0])
        for c0 in range(s0, e0, C):
            chunks.append((rid, order[c0 : min(c0 + C, e0)]))
    chunks.sort(key=lambda x: -len(x[1]))

    per_core = [[] for _ in range(NCORES)]
    counts = [0] * NCORES
    items = [0] * NCORES
    for ch in chunks:
        k = min(range(NCORES), key=lambda q: (counts[q], items[q]))
        per_core[k].append(ch)
        counts[k] += 1
        items[k] += len(ch[1])
    assert max(counts) <= G, f"chunk overflow: {counts}"

    if mode == "f32":
        def wcast(x):
            return x
        wdt = np.float32
    else:
        def wcast(x):
            return x.astype(ml_dtypes.bfloat16)
        wdt = ml_dtypes.bfloat16
    nw = 2 if mode == "bf16x2" else 1

    in_maps = []
    slot_maps = []  # per core: (slot_array, item_positions_array)
    for k in range(NCORES):
        wrows = np.zeros((nw * G, DIM * DIM), dtype=wdt)
        hidx = np.full((128, IDX_COLS), PAD_IDX, dtype=np.int32)
        tidx = np.full((128, IDX_COLS), PAD_IDX, dtype=np.int32)
        slots = []
        positions = []
        for g, (rid, pos) in enumerate(per_core[k]):
            w32 = rel_weight[rid]
            whi = wcast(w32)
            wrows[g] = whi
            if nw == 2:
                wrows[G + g] = wcast(w32 - whi.astype(np.float32))
            s = g * C + np.arange(len(pos))
            hidx[s % 128, s // 128] = h[pos]
            tidx[s % 128, s // 128] = t[pos]
            slots.append(s)
            positions.append(pos)
        slots = np.concatenate(slots) if slots else np.zeros(0, np.int64)
        positions = (
            np.concatenate(positions) if positions else np.zeros(0, np.int64)
        )
        slot_maps.append((slots, positions))
        in_maps.append(
            {"ent": ent_weight, "wrows": wrows, "hidx": hidx, "tidx": tidx}
        )
    return in_maps, slot_maps


def _run(h, r, t, ent_weight, rel_weight, trace=False, mode=None):
    if mode is None:
        mode = MODE
    h = np.asarray(h).astype(np.int64)
    r = np.asarray(r).astype(np.int64)
    t = np.asarray(t).astype(np.int64)
    ent_weight = np.ascontiguousarray(np.asarray(ent_weight, dtype=np.float32))
    rel_weight = np.ascontiguousarray(np.asarray(rel_weight, dtype=np.float32))
    assert ent_weight.shape == (N_ENT, DIM)
    assert rel_weight.shape == (N_REL, DIM * DIM)

    in_maps, slot_maps = _pack(h, r, t, ent_weight, rel_weight, mode)
    nc = _get_nc(mode)
    res = run_bass_kernel_spmd(
        nc, in_maps, core_ids=list(range(NCORES)), trace=trace
    )
    scores = np.empty(h.shape[0], dtype=np.float32)
    for k in range(NCORES):
        o = res.results[k]["out"]
        slots, positions = slot_maps[k]
        scores[positions] = o[slots % 128, slots // 128]
    return scores, res


def kernel(h, r, t, ent_weight, rel_weight):
    scores, _ = _run(h, r, t, ent_weight, rel_weight, trace=False)
    return scores
